# revision 31
# baseline (speedup 1.0000x reference)
"""Trainium2 Bass kernel for nn_CageSkinning (B=8, N=8192, 42-vert cage, 80 faces).

Sharding: pure data-parallel over batch B across the 8 NeuronCores (core b
handles batch b).

Split of work:
  host   everything O(B * small): decoder MLP (512-512-512-256-42), keypoint
         top-5 masking, influence -> new cage, and the cage-derived MVC
         constants (det-plane coefficients, cdist affine row).  These are a
         few MFLOP of numpy and moving them off-device removes ~20 MB of
         replicated weight upload per call through the axon tunnel.
  device the O(N)-heavy part only: MVC weights for 8192 points per core in
         16 chunks of 512 (entities-on-partitions; gathers as one-hot
         matmuls; sign(det(u)) via the affine form det(c_i - p) = V_f -
         n_f . p), then deformed = (W @ new_cage) / rowsum.
  guard  the 100-iter cage-shrink loop is a no-op iff every cage vertex has
         a point within 0.4 at t=0; the device computes min d^2 per vertex
         as a byproduct of the MVC cdist and returns a far-vertex count.
         If any vertex is far (not the case for typical data), the cage is
         evolved faithfully on the host and the kernel is re-run once.

Warm-call latency notes: the JAX persistent compilation cache is enabled so
repeat calls skip the walrus BIR->NEFF recompile that a fresh jit closure
would otherwise trigger inside run_bass_kernel_spmd.
"""

import os
import numpy as np

f32 = np.float32

N_CORES = 8
B, NPTS, NC, NF, NE, K = 8, 8192, 42, 80, 120, 12
N_INFLUENCE = 5
P = 512                      # points per chunk
NCHUNK = NPTS // P
EPS = 1e-8
CAGE_DIST, CAGE_ITERS, CAGE_STEP = 0.4, 100, 0.01

_CACHE = {}


def _configure_jax():
    if _CACHE.get("jax_cfg"):
        return
    import jax
    cache_dir = os.path.join(os.environ.get("TMPDIR", "/tmp"), "jax_pcc")
    try:
        jax.config.update("jax_compilation_cache_dir", cache_dir)
        jax.config.update("jax_persistent_cache_min_compile_time_secs", 0.0)
        jax.config.update("jax_persistent_cache_min_entry_size_bytes", 0)
    except Exception:
        pass
    _CACHE["jax_cfg"] = True


# ----------------------------------------------------------------------------
# host-side static structure (from the faces index tensor)
# ----------------------------------------------------------------------------
def _structure(faces):
    faces = np.asarray(faces).astype(np.int64)
    assert faces.shape == (NF, 3)
    edges = {}
    eid = np.zeros((NF, 3), np.int64)          # edge opposite vertex i
    for f in range(NF):
        for i in range(3):
            a, b = int(faces[f, (i + 1) % 3]), int(faces[f, (i + 2) % 3])
            kk = (min(a, b), max(a, b))
            if kk not in edges:
                edges[kk] = len(edges)
            eid[f, i] = edges[kk]
    assert len(edges) == NE
    edge_ab = np.zeros((NE, 2), np.int64)
    for (a, b), e in edges.items():
        edge_ab[e] = (a, b)

    C = {}
    # D matmul: D[3c+x, p] = cage[x,c] - pts[x,p];  lhsT [4,126]
    D4 = np.zeros((4, 3 * NC), f32)
    for c in range(NC):
        for x in range(3):
            D4[x, 3 * c + x] = -1.0
    C["D4"] = D4                                  # row 3 filled on device
    # sum of squares of xyz triples: [126, 42]
    S3 = np.zeros((3 * NC, NC), f32)
    for c in range(NC):
        S3[3 * c:3 * c + 3, c] = 1.0
    C["S3"] = S3
    # replicate invd (42) to 126
    R3 = np.zeros((NC, 3 * NC), f32)
    for c in range(NC):
        R3[c, 3 * c:3 * c + 3] = 1.0
    C["R3"] = R3
    # edge difference per component: [126, 120] x3
    for x in range(3):
        UE = np.zeros((3 * NC, NE), f32)
        for e, (a, b) in enumerate(edge_ab):
            UE[3 * a + x, e] += 1.0
            UE[3 * b + x, e] -= 1.0
        C[f"UE{x}"] = UE
    # per slot-tile T (40 faces each): maps
    for T in range(2):
        g = np.zeros((NE, NE), f32)
        gn = np.zeros((NE, NE), f32)
        gp = np.zeros((NE, NE), f32)
        hm = np.zeros((NE, NE), f32)
        fm = np.zeros((NF, NE), f32)
        cn = np.zeros((NE, NE), f32)
        cp = np.zeros((NE, NE), f32)
        wm = np.zeros((NE, NC), f32)
        for r in range(NE):
            f = 40 * T + r // 3
            i = r % 3
            g[eid[f, i], r] = 1.0
            gn[eid[f, (i + 1) % 3], r] = 1.0
            gp[eid[f, (i + 2) % 3], r] = 1.0
            for j in range(3):
                hm[eid[f, j], r] += 0.5 if j != i else -0.5
            fm[f, r] = 1.0
            cn[(r // 3) * 3 + (i + 1) % 3, r] = 1.0
            cp[(r // 3) * 3 + (i + 2) % 3, r] = 1.0
            wm[r, faces[f, i]] = 1.0
        C[f"G{T}"], C[f"GN{T}"], C[f"GP{T}"] = g, gn, gp
        C[f"HM{T}"], C[f"FM{T}"] = hm, fm
        C[f"CN{T}"], C[f"CP{T}"], C[f"WM{T}"] = cn, cp, wm
    # h per face: [120, 80]
    HF = np.zeros((NE, NF), f32)
    for f in range(NF):
        for j in range(3):
            HF[eid[f, j], f] += 0.5
    C["HF"] = HF
    # pre-scaled by 1/pi for the range-reduced sin path
    inv_pi = np.float64(1.0) / np.pi
    C["HM0"] = (C["HM0"].astype(np.float64) * inv_pi).astype(f32)
    C["HM1"] = (C["HM1"].astype(np.float64) * inv_pi).astype(f32)
    C["HF"] = (C["HF"].astype(np.float64) * inv_pi).astype(f32)
    import ml_dtypes
    C["FMB0"] = C["FM0"].astype(ml_dtypes.bfloat16)
    C["FMB1"] = C["FM1"].astype(ml_dtypes.bfloat16)
    C["ONESC"] = np.ones((128, 1), f32)
    # integer-valued maps are bf16-exact: store them inline as bf16 and
    # upcast once on device (halves the BIR json the jit re-serializes
    # per call)
    bf = ml_dtypes.bfloat16
    for k in ("D4", "S3", "R3", "UE0", "UE1", "UE2", "G0", "G1", "GN0",
              "GN1", "GP0", "GP1", "CN0", "CN1", "CP0", "CP1", "FM0",
              "FM1", "WM0", "WM1"):
        assert np.all(C[k] == C[k].astype(bf).astype(f32)), k
        C[k] = C[k].astype(bf)
    return C


# ----------------------------------------------------------------------------
# device program: MVC weights + deform for one batch (one core)
# ----------------------------------------------------------------------------
def _build_main(consts, loop_mult=1):
    import concourse.bacc as bacc
    import concourse.mybir as mybir
    from concourse.tile import TileContext
    from contextlib import ExitStack

    dt = mybir.dt
    AL = mybir.AluOpType
    AF = mybir.ActivationFunctionType

    nc = bacc.Bacc("TRN2", target_bir_lowering=False, debug=False,
                   num_devices=N_CORES)
    Vv, Ss, Tt, Gg = nc.vector, nc.scalar, nc.tensor, nc.gpsimd

    def din(name, shape, d=None):
        return nc.dram_tensor(name, list(shape), d or dt.float32,
                              kind="ExternalInput").ap()

    i_pts = din("i_pts", [3, NPTS], dt.float16)  # fp16 over the tunnel
    i_d4row = din("i_d4row", [1, 3 * NC])        # cage flat (xyz interleaved)
    i_det4 = din("i_det4", [4, NF])              # [-nf; Vf] det-plane coeffs
    i_nct = din("i_nct", [NC, 3])                # new cage, transposed

    # single output: cols [0, NPTS) = deformed points, col NPTS = far-vertex
    # count of the cage-shrink guard (integer 0..42, exact in fp16)
    o_def = nc.dram_tensor("o_def", [3, NPTS + 8], dt.float16,
                           kind="ExternalOutput").ap()

    cd = {k: nc.inline_tensor(v, f"c_{k}") for k, v in consts.items()}

    with TileContext(nc) as tc, ExitStack() as ctx:
        cpool = ctx.enter_context(tc.tile_pool(name="consts", bufs=1))
        spool = ctx.enter_context(tc.tile_pool(name="small", bufs=1))
        work = ctx.enter_context(tc.tile_pool(name="work", bufs=1))
        ps = ctx.enter_context(tc.tile_pool(name="psum", bufs=3, space="PSUM"))

        # bf16-stored integer-valued maps are upcast to f32 once here (their
        # matmul rhs operands are f32); FMB*/D4 are consumed as bf16 directly
        BF16_UP = {"S3", "R3", "UE0", "UE1", "UE2", "G0", "G1", "GN0", "GN1",
                   "GP0", "GP1", "CN0", "CN1", "CP0", "CP1", "FM0", "FM1",
                   "WM0", "WM1"}
        CT = {}
        for k in consts:
            if k in BF16_UP:
                stg = cpool.tile(list(consts[k].shape), dt.bfloat16,
                                 name=f"s_{k}", tag="cstage", bufs=2)
                nc.sync.dma_start(stg, cd[k].ap())
                CT[k] = cpool.tile(list(consts[k].shape), dt.float32,
                                   name=f"t_{k}")
                Vv.tensor_copy(out=CT[k], in_=stg)
            else:
                CT[k] = cpool.tile(list(consts[k].shape),
                                   dt.from_np(consts[k].dtype), name=f"t_{k}")
                nc.sync.dma_start(CT[k], cd[k].ap())

        def mm(out, lhsT, rhs, **kw):
            Tt.matmul(out, lhsT, rhs, **kw)

        def pt(rows, cols=P, name="pmm", tag="pmm"):
            t = ps.tile([128, cols], dt.float32, name=name, tag=tag,
                        bufs=(2 if tag == "pga" else 3))
            return t[0:rows, :]

        B_D4 = spool.tile([4, 3 * NC], dt.float32)
        Vv.tensor_copy(out=B_D4[0:4, :], in_=CT["D4"])
        nc.sync.dma_start(B_D4[3:4, :], i_d4row)
        B_DET4 = spool.tile([4, NF], dt.float32)
        nc.sync.dma_start(B_DET4, i_det4)
        NCT = spool.tile([NC, 3], dt.float32)
        nc.sync.dma_start(NCT, i_nct)

        eps8 = spool.tile([128, 1], dt.float32)
        Vv.memset(eps8, EPS)
        one_c = spool.tile([128, 1], dt.float32)
        Vv.memset(one_c, 1.0)
        zeroT = spool.tile([128, P], dt.float32)
        Vv.memset(zeroT, 0.0)
        # running min of d^2 across chunks (guard), and the staged output
        dacc = spool.tile([NC, P], dt.float32)
        Vv.memset(dacc, 1e30)

        # full-width [pts; 1] rhs, built once: row 3 = ones, rows 0-2 = f32
        # upcast of the fp16 points
        ptsh = spool.tile([3, NPTS + 8], dt.float16, name="ptsh", tag="big16")
        nc.sync.dma_start(ptsh[0:3, 0:NPTS], i_pts)
        RC = spool.tile([4, NPTS], dt.float32)
        Vv.memset(RC, 1.0)
        Vv.tensor_copy(out=RC[0:3, :], in_=ptsh[0:3, 0:NPTS])
        # staged output reuses ptsh's slot (dead once RC is built)
        DEF = spool.tile([3, NPTS + 8], dt.float16, name="DEF", tag="big16")
        Vv.memset(DEF, 0.0)

        for ch_ in range(NCHUNK * loop_mult):
            ch = ch_ % NCHUNK
            rc = RC[:, ch * P:(ch + 1) * P]
            D_ps = pt(3 * NC, name="p_D", tag="pga")
            mm(D_ps, B_D4, rc)
            D_sb = work.tile([3 * NC, P], dt.float32, name="D_sb", bufs=2)
            Ss.copy(D_sb, D_ps)
            DD = work.tile([3 * NC, P], dt.float32, name="DD", bufs=2)
            Ss.square(DD, D_ps)
            d2_ps = pt(NC, name="p_d2", tag="pga")
            mm(d2_ps, CT["S3"], DD)
            # guard: running min of d^2
            Vv.tensor_tensor(out=dacc, in0=dacc, in1=d2_ps, op=AL.min)

            d_t = work.tile([NC, P], dt.float32, name="d_t")
            Ss.sqrt(d_t, d2_ps)
            dpe = work.tile([NC, P], dt.float32, name="dpe", tag="xx")
            Gg.tensor_scalar(out=dpe, in0=d_t, scalar1=EPS, scalar2=None,
                             op0=AL.add)
            invd = work.tile([NC, P], dt.float32, name="invd")
            Vv.reciprocal(invd, dpe)
            ir_ps = pt(3 * NC, name="p_ir", tag="pga")
            mm(ir_ps, CT["R3"], invd)
            u_t = work.tile([3 * NC, P], dt.float32, name="u_t")
            Vv.tensor_tensor(out=u_t, in0=D_sb, in1=ir_ps, op=AL.mult)

            # edges
            l3 = work.tile([NE, 3, P], dt.float32, name="l3")
            for x in range(3):
                ue_ps = pt(NE, name="p_ue", tag="pga")
                mm(ue_ps, CT[f"UE{x}"], u_t)
                Ss.square(l3[:, x, :], ue_ps)
            l2 = work.tile([NE, P], dt.float32, name="l2")
            Vv.tensor_reduce(out=l2, in_=l3.rearrange("p a q -> p q a"),
                             axis=mybir.AxisListType.X, op=AL.add)
            xc = work.tile([NE, P], dt.float32, name="xc")
            Ss.activation(xc, l2, AF.Sqrt, scale=0.25)
            Vv.tensor_scalar(out=xc, in0=xc, scalar1=(1.0 - 1e-7), scalar2=None,
                             op0=AL.min)
            xx = work.tile([NE, P], dt.float32, name="xx")
            Ss.square(xx, xc)
            om = work.tile([NE, P], dt.float32, name="om")
            Vv.tensor_scalar(out=om, in0=xx, scalar1=-1.0, scalar2=1.0,
                             op0=AL.mult, op1=AL.add)
            sq = work.tile([NE, P], dt.float32, name="sq")
            Ss.sqrt(sq, om)
            sq1 = work.tile([NE, P], dt.float32, name="sq1")
            Gg.tensor_scalar(out=sq1, in0=sq, scalar1=1.0, scalar2=None,
                             op0=AL.add)
            rcp = work.tile([NE, P], dt.float32, name="rcp")
            Vv.reciprocal(rcp, sq1)
            tt_ = work.tile([NE, P], dt.float32, name="tt_")
            Vv.tensor_tensor(out=tt_, in0=xc, in1=rcp, op=AL.mult)
            the = work.tile([NE, P], dt.float32, name="the", bufs=2)
            Ss.activation(the, tt_, AF.Arctan)
            Gg.tensor_scalar(out=the, in0=the, scalar1=4.0, scalar2=None,
                             op0=AL.mult)
            sin_e = work.tile([NE, P], dt.float32, name="sin_e")
            Vv.scalar_tensor_tensor(out=sin_e, in0=xc, scalar=2.0, in1=sq,
                                    op0=AL.mult, op1=AL.mult)
            # det sign (affine in p); bf16 exact for +-1/0
            det_ps = pt(NF, name="p_det", tag="pga")
            mm(det_ps, B_DET4, rc)
            sgnf = work.tile([NF, P], dt.bfloat16, name="sgnf")
            Ss.sign(sgnf, det_ps)
            # stacked (h-theta)/pi (both tiles) and h/pi (faces); then one
            # range-reduced sin chain: k=round(t), r=t-k, sin = sin(pi r)(1-2k^2)
            SIN3 = ps.tile([128, 3, P], dt.float32, name="p_sin3", tag="pwide",
                           bufs=1)
            mm(SIN3[0:NE, 0, :], CT["HM0"], the)
            mm(SIN3[0:NE, 1, :], CT["HM1"], the)
            mm(SIN3[0:NF, 2, :], CT["HF"], the)
            # range-reduced sin on t in [0, 1.5): k = (t>=1), sin(pi t) =
            # sin(pi (t-k)) * (1-2k)
            tcl = work.tile([NE, 3, P], dt.float32, name="tcl", tag="w6a")
            Vv.tensor_scalar(out=tcl, in0=SIN3[0:NE, :, :], scalar1=1.4999,
                             scalar2=None, op0=AL.min)
            kf = work.tile([NE, 3, P], dt.float32, name="kf", tag="w6c")
            Vv.tensor_scalar(out=kf, in0=tcl, scalar1=1.0, scalar2=None,
                             op0=AL.is_ge)
            r_ = work.tile([NE, 3, P], dt.float32, name="r_", tag="l3")
            Vv.tensor_tensor(out=r_, in0=tcl, in1=kf, op=AL.subtract)
            kk = work.tile([NE, 3, P], dt.float32, name="kk", tag="w6a")
            Gg.tensor_scalar(out=kk, in0=kf, scalar1=-2.0, scalar2=1.0,
                             op0=AL.mult, op1=AL.add)
            sinr = work.tile([NE, 3, P], dt.float32, name="sinr", tag="w6c")
            Ss.activation(sinr, r_, AF.Sin, scale=float(np.pi))
            sinall = work.tile([NE, 3, P], dt.float32, name="sinall", tag="w6b")
            Vv.tensor_tensor(out=sinall, in0=sinr, in1=kk, op=AL.mult)
            # 1/d for the factored-out df denominator term
            rd = work.tile([NC, P], dt.float32, name="rd")
            Vv.reciprocal(rd, d_t)

            wts = []
            for T in range(2):
                th_ps = pt(NE, name="p_th")
                mm(th_ps, CT[f"G{T}"], the)
                tn_ps = pt(NE, name="p_tn")
                mm(tn_ps, CT[f"GN{T}"], the)
                tp_ps = pt(NE, name="p_tp")
                mm(tp_ps, CT[f"GP{T}"], the)
                tn_sb = work.tile([NE, P], dt.float32, name=f"tn{T}")
                Ss.copy(tn_sb, tn_ps)
                tp_sb = work.tile([NE, P], dt.float32, name=f"tp{T}")
                Ss.copy(tp_sb, tp_ps)
                sn_ps = pt(NE, name="p_sn")
                mm(sn_ps, CT[f"GN{T}"], sin_e)
                sinn = work.tile([NE, P], dt.float32, name=f"sinn{T}")
                Ss.copy(sinn, sn_ps)
                sp_ps = pt(NE, name="p_sp")
                mm(sp_ps, CT[f"GP{T}"], sin_e)
                sinp = work.tile([NE, P], dt.float32, name=f"sinp{T}")
                Ss.copy(sinp, sp_ps)
                sinhm = sinall[:, T, :]
                shf_ps = pt(NE, name="p_shf")
                mm(shf_ps, CT[f"FM{T}"], sinall[0:NF, 2, :])

                denc = work.tile([NE, P], dt.float32, name=f"dnc{T}")
                Vv.tensor_tensor(out=denc, in0=sinn, in1=sinp, op=AL.mult)
                Gg.tensor_scalar(out=denc, in0=denc, scalar1=EPS, scalar2=None,
                                 op0=AL.add)
                rdc = work.tile([NE, P], dt.float32, name=f"rdc{T}")
                Vv.reciprocal(rdc, denc)
                t1 = work.tile([NE, P], dt.float32, name=f"t1{T}")
                Vv.tensor_tensor(out=t1, in0=shf_ps, in1=sinhm, op=AL.mult)
                c_t = work.tile([NE, P], dt.float32, name=f"c{T}")
                Vv.scalar_tensor_tensor(out=c_t, in0=t1, scalar=2.0, in1=rdc,
                                        op0=AL.mult, op1=AL.mult)
                Gg.tensor_scalar(out=c_t, in0=c_t, scalar1=-1.0, scalar2=None,
                                 op0=AL.add)
                om2 = work.tile([NE, P], dt.float32, name=f"om2{T}")
                Ss.square(om2, c_t)
                Ss.activation(om2, om2, AF.Relu, bias=one_c[0:NE, :],
                              scale=-1.0)
                smag = work.tile([NE, P], dt.float32, name=f"smag{T}")
                Ss.sqrt(smag, om2)
                sgn_ps = pt(NE, name="p_sgn")
                Tt.matmul(sgn_ps, CT[f"FMB{T}"], sgnf)
                s_t = work.tile([NE, P], dt.float32, name=f"s{T}")
                Vv.tensor_tensor(out=s_t, in0=sgn_ps, in1=smag, op=AL.mult)
                sprv_ps = pt(NE, name="p_sprv")
                mm(sprv_ps, CT[f"CP{T}"], s_t)
                den = work.tile([NE, P], dt.float32, name=f"den{T}")
                Vv.tensor_tensor(out=den, in0=sinn, in1=sprv_ps, op=AL.mult)
                cn_ps = pt(NE, name="p_cn")
                mm(cn_ps, CT[f"CN{T}"], c_t)
                cp_ps = pt(NE, name="p_cp")
                mm(cp_ps, CT[f"CP{T}"], c_t)
                n1 = work.tile([NE, P], dt.float32, name=f"n1{T}")
                Vv.tensor_tensor(out=n1, in0=cn_ps, in1=tp_sb, op=AL.mult)
                n2 = work.tile([NE, P], dt.float32, name=f"n2{T}")
                Vv.tensor_tensor(out=n2, in0=th_ps, in1=n1, op=AL.subtract)
                n3 = work.tile([NE, P], dt.float32, name=f"n3{T}", tag=f"n1{T}")
                Vv.tensor_tensor(out=n3, in0=cp_ps, in1=tn_sb, op=AL.mult)
                Vv.tensor_tensor(out=n2, in0=n2, in1=n3, op=AL.subtract)
                rdn = work.tile([NE, P], dt.float32, name=f"rdn{T}")
                Vv.reciprocal(rdn, den)
                w_t = work.tile([NE, P], dt.float32, name=f"w{T}", bufs=2)
                Vv.tensor_tensor(out=w_t, in0=n2, in1=rdn, op=AL.mult)
                asp = work.tile([NE, P], dt.float32, name=f"asp{T}",
                                tag=f"n1{T}")
                Ss.activation(asp, sprv_ps, AF.Abs)
                msp = work.tile([NE, P], dt.uint8, name=f"msp{T}")
                Vv.tensor_scalar(out=msp, in0=asp, scalar1=1e-6, scalar2=None,
                                 op0=AL.is_lt)
                Vv.copy_predicated(out=w_t, mask=msp, data=zeroT[0:NE, :])
                wts.append(w_t)

            Wp_ps = pt(NC, name="p_W", tag="pga")
            mm(Wp_ps, CT["WM0"], wts[0], start=True, stop=False)
            mm(Wp_ps, CT["WM1"], wts[1], start=False, stop=True)
            W_sb = work.tile([NC, P], dt.float32, name="W_sb", bufs=2)
            Vv.tensor_tensor(out=W_sb, in0=Wp_ps, in1=rd, op=AL.mult)
            rs_ps = pt(1, name="p_rs", tag="pga")
            mm(rs_ps, CT["ONESC"][0:NC, 0:1], W_sb)
            du_ps = pt(3, name="p_du", tag="pga")
            mm(du_ps, NCT, W_sb)
            rsi = work.tile([1, P], dt.float32, name="rsi", bufs=2)
            Ss.activation(rsi, rs_ps, AF.Identity, bias=eps8[0:1, :])
            Vv.reciprocal(rsi, rsi)
            rsi3 = work.tile([3, P], dt.float32, name="rsi3", bufs=2)
            Gg.partition_broadcast(rsi3, rsi, channels=3)
            Vv.tensor_tensor(out=DEF[:, ch * P:(ch + 1) * P], in0=du_ps,
                             in1=rsi3, op=AL.mult)

        # guard tail: fold the running d^2 min, count far vertices, stash the
        # count in the output's spare column, then one DMA for everything
        mind2 = spool.tile([NC, 1], dt.float32)
        Vv.tensor_reduce(out=mind2, in_=dacc, axis=mybir.AxisListType.X,
                         op=AL.min)
        mroot = spool.tile([NC, 1], dt.float32)
        Ss.sqrt(mroot, mind2)
        far = spool.tile([NC, 1], dt.float32)
        Vv.tensor_scalar(out=far, in0=mroot, scalar1=CAGE_DIST, scalar2=None,
                         op0=AL.is_gt)
        nf_ps = pt(1, 1, name="p_nf")
        Tt.matmul(nf_ps, far, CT["ONESC"][0:NC, 0:1])
        Ss.copy(DEF[0:1, NPTS:NPTS + 1], nf_ps)
        nc.sync.dma_start(o_def, DEF)

    nc.finalize()
    return nc


# ----------------------------------------------------------------------------
# host math: decoder MLP, top-k masking, new cage, det-plane constants
# ----------------------------------------------------------------------------
def _host_small(inputs, cages):
    """Per-batch small tensors from the (possibly evolved) cages.

    cages: (B, 3, NC) f32.  Returns per-core input maps (sans points)."""
    faces = np.asarray(inputs["faces"]).astype(np.int64)
    sf = np.asarray(inputs["source_f"], f32)
    tf = np.asarray(inputs["target_f"], f32)
    x = np.concatenate([sf, tf], axis=1)                          # (B,512)
    W1 = np.asarray(inputs["W1"], f32)
    W2 = np.asarray(inputs["W2"], f32)
    W3 = np.asarray(inputs["W3"], f32)
    W4 = np.asarray(inputs["W4"], f32)
    b1 = np.asarray(inputs["b1"], f32)
    b2 = np.asarray(inputs["b2"], f32)
    b3 = np.asarray(inputs["b3"], f32)
    b4 = np.asarray(inputs["b4"], f32)
    h = np.maximum(x @ W1 + b1, 0.0).astype(f32)
    h = np.maximum(h @ W2 + b2, 0.0).astype(f32)
    h = np.maximum(h @ W3 + b3, 0.0).astype(f32)
    ioff = (h @ W4 + b4).astype(f32)                              # (B,42)
    ip = np.asarray(inputs["influence_param"], f32)               # (K,42)
    kps = np.asarray(inputs["source_keypoints"], f32)             # (B,K,3)
    kpt = np.asarray(inputs["target_keypoints"], f32)

    maps = []
    for b in range(B):
        cage = np.ascontiguousarray(cages[b])                     # (3,42)
        # keypoint->cage distances, 5th-smallest threshold, mask
        diff = kps[b][:, :, None] - cage[None, :, :]              # (K,3,NC)
        dist = np.sum(diff * diff, axis=1).astype(f32)            # (K,NC)
        thr = np.partition(dist, N_INFLUENCE - 1, axis=1)[:, N_INFLUENCE - 1:N_INFLUENCE]
        keep = (dist <= thr).astype(f32)
        infl = ((ip + ioff[b][None, :]) * keep).astype(f32)       # (K,NC)
        dk = (kpt[b] - kps[b]).T.astype(f32)                      # (3,K)
        new_cage = (cage + dk @ infl).astype(f32)                 # (3,42)

        # det-plane constants: det(c0-p, c1-p, c2-p) = Vf - nf.p
        cageT = cage.T                                            # (NC,3)
        A_ = cageT[faces[:, 0]]                                   # (NF,3)
        B_ = cageT[faces[:, 1]]
        C_ = cageT[faces[:, 2]]
        cBC = np.cross(B_, C_).astype(f32)
        cAC = np.cross(A_, C_).astype(f32)
        cAB = np.cross(A_, B_).astype(f32)
        nf_ = (cBC - cAC + cAB).astype(f32)
        vf = np.sum(A_ * cBC, axis=1).astype(f32)                 # (NF,)
        det4 = np.empty((4, NF), f32)
        det4[0:3] = -nf_.T
        det4[3] = vf

        maps.append({
            "i_d4row": np.ascontiguousarray(cage.T.reshape(1, 3 * NC)),
            "i_det4": det4,
            "i_nct": np.ascontiguousarray(new_cage.T),
        })
    return maps


def _evolve_cages_host(cages, pts):
    """Faithful 100-iter cage shrink, vectorized per vertex on the host.

    cages: (B,3,NC) f32; pts: (B,3,NPTS) f32.  Matches the reference's
    per-iteration f32 update c + 0.01*(-c)*upd with distances recomputed
    from the current cage each iteration (each vertex only depends on its
    own position, so per-vertex simulation is exact)."""
    out = []
    for b in range(B):
        c = cages[b].copy()                                       # (3,NC)
        p = pts[b]                                                # (3,NPTS)
        pp = np.sum(p.astype(np.float64) ** 2, axis=0)
        for _ in range(CAGE_ITERS):
            # min_j ||c_v - p_j|| per vertex, f32-faithful enough: use f64
            # accumulation for the distance (comparison against 0.4 is far
            # from ties for generic data)
            cc = np.sum(c.astype(np.float64) ** 2, axis=0)        # (NC,)
            cross = c.astype(np.float64).T @ p.astype(np.float64)  # (NC,NPTS)
            d2 = cc[:, None] - 2.0 * cross + pp[None, :]
            mind = np.sqrt(np.maximum(d2.min(axis=1), 0.0)).astype(f32)
            upd = (mind > CAGE_DIST).astype(f32)                  # (NC,)
            if not upd.any():
                break
            c = (c + (CAGE_STEP * (-c)) * upd[None, :]).astype(f32)
        out.append(c)
    return out


def kernel(**inputs):
    _configure_jax()
    from concourse.bass_utils import run_bass_kernel_spmd

    faces = np.asarray(inputs["faces"])
    key = faces.tobytes()
    if ("main", key) not in _CACHE:
        consts = _structure(faces)
        _CACHE[("main", key)] = _build_main(consts)
    nc = _CACHE[("main", key)]

    src = np.ascontiguousarray(np.asarray(inputs["source_shape"], f32))
    srch = src.astype(np.float16)
    cage0 = np.asarray(inputs["cage_v"], f32)[0]                  # (3,42)
    cages = [cage0.copy() for _ in range(B)]

    small = _host_small(inputs, cages)
    maps = [{"i_pts": srch[b], **small[b]} for b in range(B)]
    res = run_bass_kernel_spmd(nc, maps, core_ids=list(range(N_CORES)))
    kernel._last = res

    nfar = np.array([res.results[b]["o_def"][0, NPTS] for b in range(B)])
    if np.any(nfar > 0):
        # rare path: cage-shrink loop is not a no-op; evolve on host, redo
        cages = _evolve_cages_host(cages, src)
        small = _host_small(inputs, cages)
        maps = [{"i_pts": srch[b], **small[b]} for b in range(B)]
        res = run_bass_kernel_spmd(nc, maps, core_ids=list(range(N_CORES)))
        kernel._last = res

    out = np.stack([res.results[b]["o_def"][:, :NPTS] for b in range(B)],
                   axis=0)
    return out.astype(np.float32)
